# revision 24
# baseline (speedup 1.0000x reference)
"""Euclidean distance layer on 8 Trainium2 NeuronCores.

out[b, o] = || x[b, :] - weight[:, o] ||_2
x: [512, 256] f32, weight: [256, 1024] f32 -> out: [512, 1024] f32

Sharding: tensor-parallel over output features (8 x 128 columns per core).

Transposed-psum fp8 design: psum[o_local, b] with k=256 contraction:

  ps  [o, b] = sum_k  w[k,o] * x[k,b]        (1 DR mm, lhsT=w)
             + sum_k (-0.5) * xsq[k,b]       (1 DR mm, lhsT=const -0.5)
  ps_w[o, 0] = sum_k  wsq[k,o] * 1           (1 DR mm, n=1)
  out [o, b] = sqrt(-2*ps + bias=wcol)       (1 ACT, f16 out)

Inputs fp8 e4m3, contiguous per partition. x k-chunks ride the sync
queue (two DMAs, 512B rows); w rides the gpsimd queue in parallel.
Squares are split: scalar-ACT takes b-half A of each x-chunk plus the
w-square (Square/Sqrt share act table set 3; the dummy sqrt is scalar's
first instruction so exactly one 1.28us table load lands at block start,
hidden under the input DMA), DVE takes b-half B of each chunk plus the
||w||^2 psum->SBUF copy. PE overlaps the main DR matmul with the square
wave. Output DMA issues from scalar with no completion wait; the fixed
~7.8us NEFF epilogue (full semaphore-file reset) outlasts the transfer.
Host work: layout/dtype prep + transpose/concat only.
"""

from contextlib import ExitStack

import numpy as np

B = 512      # batch
BH = B // 2  # b-half
K = 256      # inputSize (contraction dim)
NOUT = 1024  # outputSize
NCORES = 8
NLOC = NOUT // NCORES  # 128 output features per core
P = 128                # partitions
KT = K // P            # 2 contraction chunks

_NC = None  # cached compiled Bass program (same SPMD program on all cores)


def _build():
    import concourse.bass as bass
    from concourse import bacc, mybir

    f32 = mybir.dt.float32
    f16 = mybir.dt.float16
    f8 = mybir.dt.float8e4
    DR = mybir.MatmulPerfMode.DoubleRow
    Sqrt = mybir.ActivationFunctionType.Sqrt
    Square = mybir.ActivationFunctionType.Square

    nc = bacc.Bacc(
        "TRN2", target_bir_lowering=False, debug=False, num_devices=NCORES
    )

    xc0 = nc.dram_tensor("xc0", [P, B], f8, kind="ExternalInput")
    xc1 = nc.dram_tensor("xc1", [P, B], f8, kind="ExternalInput")
    wh = nc.dram_tensor("wh", [P, KT, NLOC], f8, kind="ExternalInput")
    out = nc.dram_tensor("out", [P, B], f16, kind="ExternalOutput")

    with ExitStack() as ctx:
        e = ctx.enter_context
        xh_sb = e(nc.sbuf_tensor("xh_sb", [P, KT, B], f8))
        wh_sb = e(nc.sbuf_tensor("wh_sb", [P, KT, NLOC], f8))
        xsq = e(nc.sbuf_tensor("xsq", [P, KT, B], f8))
        wlsq = e(nc.sbuf_tensor("wlsq", [P, KT, NLOC], f8))
        neghalf = e(nc.sbuf_tensor("neghalf", [P, KT, NLOC], f8))
        ones1 = e(nc.sbuf_tensor("ones1", [P, KT, 1], f8))
        wcol = e(nc.sbuf_tensor("wcol", [P, 1], f32))
        out_sb = e(nc.sbuf_tensor("out_sb", [P, B], f16))
        dumm = e(nc.sbuf_tensor("dumm", [1, 1], f32))

        ps = e(nc.psum_tensor("ps", [P, B], f32))       # one full bank
        ps_w = e(nc.psum_tensor("ps_w", [P, 1], f32))   # ||w||^2 column

        s_inx = [e(nc.semaphore(f"s_inx{c}")) for c in range(KT)]
        s_inw = e(nc.semaphore("s_inw"))
        s_wsq = e(nc.semaphore("s_wsq"))
        s_sq = e(nc.semaphore("s_sq"))      # 4 = all quarter-squares done
        s_mm = e(nc.semaphore("s_mm"))      # 1 = ps_w, 2 = ps done
        s_wcol = e(nc.semaphore("s_wcol"))
        s_sqrt = e(nc.semaphore("s_sqrt"))
        s_out = e(nc.semaphore("s_out"))    # inc only; no waiter
        s_dum = e(nc.semaphore("s_dum"))

        block = e(nc.Block())

        @block.sync
        def _(sync):
            sync.dma_start(
                out=xh_sb[:, 0, :], in_=xc0[:, :]
            ).then_inc(s_inx[0], 16)
            sync.dma_start(
                out=xh_sb[:, 1, :], in_=xc1[:, :]
            ).then_inc(s_inx[1], 16)

        @block.gpsimd
        def _(gpsimd):
            gpsimd.dma_start(
                out=wh_sb[:, :, :], in_=wh[:, :, :]
            ).then_inc(s_inw, 16)


        @block.scalar
        def _(scalar):
            # dummy sqrt FIRST: exactly one act-table load (set 3 covers
            # Sqrt and Square), hoisted to block start
            scalar.wait_ge(s_dum, 1)
            scalar.activation(dumm[:, :], dumm[:, :], Sqrt)
            # square b-half A of each chunk as it lands (DVE takes half B)
            scalar.wait_ge(s_inx[0], 16)
            scalar.activation(
                xsq[:, 0, 0:BH], xh_sb[:, 0, 0:BH], Square
            ).then_inc(s_sq)
            scalar.wait_ge(s_inw, 16)
            scalar.activation(
                wlsq[:, :, :], wh_sb[:, :, :], Square
            ).then_inc(s_wsq)
            scalar.wait_ge(s_inx[1], 16)
            scalar.activation(
                xsq[:, 1, 0:BH], xh_sb[:, 1, 0:BH], Square
            ).then_inc(s_sq)
            scalar.wait_ge(s_mm, 2)
            scalar.wait_ge(s_wcol, 1)
            scalar.activation(
                out_sb[:, :], ps[:, :], Sqrt, bias=wcol[:, :], scale=-2.0
            ).then_inc(s_sqrt)
            scalar.wait_ge(s_sqrt, 1)
            scalar.dma_start(
                out=out[:, :], in_=out_sb[:, :]
            ).then_inc(s_out, 16)
            # no completion wait: the fixed NEFF epilogue outlasts the
            # transfer; nrt reads outputs only after full teardown.

        @block.vector
        def _(vector):
            vector.memset(dumm[:, :], 1.0).then_inc(s_dum)
            vector.memset(neghalf[:, :, :], -0.5)
            vector.memset(ones1[:, :, :], 1.0)
            vector.wait_ge(s_inx[0], 16)
            vector.tensor_mul(
                xsq[:, 0, BH:B], xh_sb[:, 0, BH:B], xh_sb[:, 0, BH:B]
            ).then_inc(s_sq)
            vector.wait_ge(s_inx[1], 16)
            vector.tensor_mul(
                xsq[:, 1, BH:B], xh_sb[:, 1, BH:B], xh_sb[:, 1, BH:B]
            ).then_inc(s_sq)
            vector.wait_ge(s_mm, 1)
            vector.tensor_copy(wcol[:, :], ps_w[:, :]).then_inc(s_wcol)

        @block.tensor
        def _(tensor):
            # main x.w per k-chunk, starting at chunk0 landing
            tensor.wait_ge(s_inx[0], 16)
            tensor.matmul(
                ps[:, :], lhsT=wh_sb[:, 0, :], rhs=xh_sb[:, 0, :],
                start=True, stop=False, skip_group_check=True,
            )
            tensor.wait_ge(s_inx[1], 16)
            tensor.matmul(
                ps[:, :], lhsT=wh_sb[:, 1, :], rhs=xh_sb[:, 1, :],
                start=False, stop=False, skip_group_check=True,
            )
            # ||w||^2 column (n=1, slots into the main stream's shadow)
            tensor.wait_ge(s_wsq, 1)
            tensor.matmul(
                ps_w[:, :], lhsT=wlsq[:, :, :], rhs=ones1[:, :, :],
                start=True, stop=True, perf_mode=DR, skip_group_check=True,
            ).then_inc(s_mm)  # = 1
            # -0.5*||x||^2 (DR, both chunks, full width)
            tensor.wait_ge(s_sq, 4)
            tensor.matmul(
                ps[:, :], lhsT=neghalf[:, :, :], rhs=xsq[:, :, :],
                start=False, stop=True, perf_mode=DR, skip_group_check=True,
            ).then_inc(s_mm)  # = 2

    nc.compile()
    return nc


def _get_nc():
    global _NC
    if _NC is None:
        _NC = _build()
    return _NC


def _np_f8():
    from concourse import mybir

    return mybir.dt.np(mybir.dt.float8e4)


def _make_in_maps(x: np.ndarray, weight: np.ndarray):
    f8 = _np_f8()
    xf = x.astype(f8)
    wf = weight.astype(f8)
    # xh[p, c, b] = x[b, c*128+p]
    xh = xf.T.reshape(KT, P, B)
    xc0 = np.ascontiguousarray(xh[0])
    xc1 = np.ascontiguousarray(xh[1])
    maps = []
    for c in range(NCORES):
        wl = wf[:, c * NLOC : (c + 1) * NLOC]  # [256, 128]
        whc = np.ascontiguousarray(wl.reshape(KT, P, NLOC).transpose(1, 0, 2))
        maps.append({"xc0": xc0, "xc1": xc1, "wh": whc})
    return maps


def run(x: np.ndarray, weight: np.ndarray, trace: bool = False):
    """Returns (full_output, BassKernelResults)."""
    from concourse.bass_utils import run_bass_kernel_spmd

    nc = _get_nc()
    res = run_bass_kernel_spmd(
        nc, _make_in_maps(x, weight), core_ids=list(range(NCORES)), trace=trace
    )
    # out[o_local, b] per core -> full [B, NOUT] f32
    full = np.concatenate(
        [res.results[c]["out"].T.astype(np.float32) for c in range(NCORES)],
        axis=1,
    )
    return full, res


def kernel(x: np.ndarray, weight: np.ndarray) -> np.ndarray:
    return run(x, weight)[0]


# revision 26
# speedup vs baseline: 1.1147x; 1.1147x over previous
"""Euclidean distance layer on 8 Trainium2 NeuronCores.

out[b, o] = || x[b, :] - weight[:, o] ||_2
x: [512, 256] f32, weight: [256, 1024] f32 -> out: [512, 1024] f32

Sharding: tensor-parallel over output features (8 x 128 columns per core).

Transposed-psum fp8 design: psum[o_local, b] with k=256 contraction:

  ps  [o, b] = sum_k  w[k,o] * x[k,b]        (1 DR mm, lhsT=w)
             + sum_k (-0.5) * xsq[k,b]       (1 DR mm, lhsT=const -0.5)
  ps_w[o, 0] = sum_k  wsq[k,o] * 1           (1 DR mm, n=1)
  out [o, b] = sqrt(-2*ps + bias=wcol)       (1 ACT, f16 out)

Inputs fp8 e4m3, contiguous per partition. x k-chunks ride the sync
queue (two DMAs, 512B rows); w rides the gpsimd queue in parallel.
Squares are split: scalar-ACT takes b-half A of each x-chunk plus the
w-square (Square/Sqrt share act table set 3; the dummy sqrt is scalar's
first instruction so exactly one 1.28us table load lands at block start,
hidden under the input DMA), DVE takes b-half B of each chunk plus the
||w||^2 psum->SBUF copy. PE overlaps the main DR matmul with the square
wave. Output DMA issues from scalar with no completion wait; the fixed
~7.8us NEFF epilogue (full semaphore-file reset) outlasts the transfer.
Host work: layout/dtype prep + transpose/concat only.
"""

from contextlib import ExitStack

import numpy as np

B = 512      # batch
BH = B // 2  # b-half
K = 256      # inputSize (contraction dim)
NOUT = 1024  # outputSize
NCORES = 8
NLOC = NOUT // NCORES  # 128 output features per core
P = 128                # partitions
KT = K // P            # 2 contraction chunks

_NC = None  # cached compiled Bass program (same SPMD program on all cores)


def _build():
    import concourse.bass as bass
    from concourse import bacc, mybir

    f32 = mybir.dt.float32
    f16 = mybir.dt.float16
    f8 = mybir.dt.float8e4
    DR = mybir.MatmulPerfMode.DoubleRow
    Sqrt = mybir.ActivationFunctionType.Sqrt
    Square = mybir.ActivationFunctionType.Square

    nc = bacc.Bacc(
        "TRN2", target_bir_lowering=False, debug=False, num_devices=NCORES
    )

    xc0 = nc.dram_tensor("xc0", [P, B], f8, kind="ExternalInput")
    xc1 = nc.dram_tensor("xc1", [P, B], f8, kind="ExternalInput")
    wh = nc.dram_tensor("wh", [P, KT, NLOC], f8, kind="ExternalInput")
    out = nc.dram_tensor("out", [P, B], f16, kind="ExternalOutput")

    with ExitStack() as ctx:
        e = ctx.enter_context
        xh_sb = e(nc.sbuf_tensor("xh_sb", [P, KT, B], f8))
        wh_sb = e(nc.sbuf_tensor("wh_sb", [P, KT, NLOC], f8))
        xsq = e(nc.sbuf_tensor("xsq", [P, KT, B], f8))
        wlsq = e(nc.sbuf_tensor("wlsq", [P, KT, NLOC], f8))
        neghalf = e(nc.sbuf_tensor("neghalf", [P, KT, NLOC], f8))
        ones1 = e(nc.sbuf_tensor("ones1", [P, KT, 1], f8))
        wcol = e(nc.sbuf_tensor("wcol", [P, 1], f32))
        out_sb = e(nc.sbuf_tensor("out_sb", [P, B], f16))
        dumm = e(nc.sbuf_tensor("dumm", [1, 1], f32))

        ps = e(nc.psum_tensor("ps", [P, B], f32))       # one full bank
        ps_w = e(nc.psum_tensor("ps_w", [P, 1], f32))   # ||w||^2 column

        s_inx = [e(nc.semaphore(f"s_inx{c}")) for c in range(KT)]
        s_inw = e(nc.semaphore("s_inw"))
        s_wsq = e(nc.semaphore("s_wsq"))
        s_sq = e(nc.semaphore("s_sq"))      # 4 = all quarter-squares done
        s_mm = e(nc.semaphore("s_mm"))      # 1 = ps_w, 2 = ps done
        s_wcol = e(nc.semaphore("s_wcol"))
        s_sqrt = e(nc.semaphore("s_sqrt"))
        s_out = e(nc.semaphore("s_out"))    # inc only; no waiter
        s_dum = e(nc.semaphore("s_dum"))
        s_cst = e(nc.semaphore("s_cst"))

        block = e(nc.Block())

        @block.sync
        def _(sync):
            sync.dma_start(
                out=xh_sb[:, 0, :], in_=xc0[:, :]
            ).then_inc(s_inx[0], 16)
            sync.dma_start(
                out=xh_sb[:, 1, :], in_=xc1[:, :]
            ).then_inc(s_inx[1], 16)

        @block.gpsimd
        def _(gpsimd):
            gpsimd.dma_start(
                out=wh_sb[:, :, :], in_=wh[:, :, :]
            ).then_inc(s_inw, 16)


        @block.scalar
        def _(scalar):
            # dummy sqrt FIRST: exactly one act-table load (set 3 covers
            # Sqrt and Square), hoisted to block start
            scalar.wait_ge(s_dum, 1)
            scalar.activation(dumm[:, :], dumm[:, :], Sqrt)
            # square b-half A of each chunk as it lands (DVE takes half B)
            scalar.wait_ge(s_inx[0], 16)
            scalar.activation(
                xsq[:, 0, 0:BH], xh_sb[:, 0, 0:BH], Square
            ).then_inc(s_sq)
            scalar.wait_ge(s_inw, 16)
            scalar.activation(
                wlsq[:, :, :], wh_sb[:, :, :], Square
            ).then_inc(s_wsq)
            scalar.wait_ge(s_inx[1], 16)
            scalar.activation(
                xsq[:, 1, 0:BH], xh_sb[:, 1, 0:BH], Square
            ).then_inc(s_sq)
            scalar.wait_ge(s_mm, 2)
            scalar.wait_ge(s_wcol, 1)
            scalar.activation(
                out_sb[:, :], ps[:, :], Sqrt, bias=wcol[:, :], scale=-2.0
            ).then_inc(s_sqrt)
            scalar.wait_ge(s_sqrt, 1)
            scalar.dma_start(
                out=out[:, :], in_=out_sb[:, :]
            ).then_inc(s_out, 16)
            # no completion wait: the fixed NEFF epilogue outlasts the
            # transfer; nrt reads outputs only after full teardown.

        @block.vector
        def _(vector):
            vector.memset(dumm[:, :], 1.0).then_inc(s_dum)
            vector.memset(neghalf[:, :, :], -0.5)
            vector.memset(ones1[:, :, :], 1.0).then_inc(s_cst)
            vector.wait_ge(s_inx[0], 16)
            vector.tensor_mul(
                xsq[:, 0, BH:B], xh_sb[:, 0, BH:B], xh_sb[:, 0, BH:B]
            ).then_inc(s_sq)
            vector.wait_ge(s_inx[1], 16)
            vector.tensor_mul(
                xsq[:, 1, BH:B], xh_sb[:, 1, BH:B], xh_sb[:, 1, BH:B]
            ).then_inc(s_sq)
            vector.wait_ge(s_mm, 1)
            vector.tensor_copy(wcol[:, :], ps_w[:, :]).then_inc(s_wcol)

        @block.tensor
        def _(tensor):
            # main x.w per k-chunk, starting at chunk0 landing
            tensor.wait_ge(s_inw, 16)
            tensor.wait_ge(s_inx[0], 16)
            tensor.matmul(
                ps[:, :], lhsT=wh_sb[:, 0, :], rhs=xh_sb[:, 0, :],
                start=True, stop=False, skip_group_check=True,
            )
            tensor.wait_ge(s_inx[1], 16)
            tensor.matmul(
                ps[:, :], lhsT=wh_sb[:, 1, :], rhs=xh_sb[:, 1, :],
                start=False, stop=False, skip_group_check=True,
            )
            # ||w||^2 column (n=1, slots into the main stream's shadow)
            tensor.wait_ge(s_cst, 1)
            tensor.wait_ge(s_wsq, 1)
            tensor.matmul(
                ps_w[:, :], lhsT=wlsq[:, :, :], rhs=ones1[:, :, :],
                start=True, stop=True, perf_mode=DR, skip_group_check=True,
            ).then_inc(s_mm)  # = 1
            # -0.5*||x||^2 (DR, both chunks, full width)
            tensor.wait_ge(s_sq, 4)
            tensor.matmul(
                ps[:, :], lhsT=neghalf[:, :, :], rhs=xsq[:, :, :],
                start=False, stop=True, perf_mode=DR, skip_group_check=True,
            ).then_inc(s_mm)  # = 2

    nc.compile()
    return nc


def _get_nc():
    global _NC
    if _NC is None:
        _NC = _build()
    return _NC


def _np_f8():
    from concourse import mybir

    return mybir.dt.np(mybir.dt.float8e4)


def _make_in_maps(x: np.ndarray, weight: np.ndarray):
    f8 = _np_f8()
    xf = x.astype(f8)
    wf = weight.astype(f8)
    # xh[p, c, b] = x[b, c*128+p]
    xh = xf.T.reshape(KT, P, B)
    xc0 = np.ascontiguousarray(xh[0])
    xc1 = np.ascontiguousarray(xh[1])
    maps = []
    for c in range(NCORES):
        wl = wf[:, c * NLOC : (c + 1) * NLOC]  # [256, 128]
        whc = np.ascontiguousarray(wl.reshape(KT, P, NLOC).transpose(1, 0, 2))
        maps.append({"xc0": xc0, "xc1": xc1, "wh": whc})
    return maps


def run(x: np.ndarray, weight: np.ndarray, trace: bool = False):
    """Returns (full_output, BassKernelResults)."""
    from concourse.bass_utils import run_bass_kernel_spmd

    nc = _get_nc()
    res = run_bass_kernel_spmd(
        nc, _make_in_maps(x, weight), core_ids=list(range(NCORES)), trace=trace
    )
    # out[o_local, b] per core -> full [B, NOUT] f32
    full = np.concatenate(
        [res.results[c]["out"].T.astype(np.float32) for c in range(NCORES)],
        axis=1,
    )
    return full, res


def kernel(x: np.ndarray, weight: np.ndarray) -> np.ndarray:
    return run(x, weight)[0]
